# revision 19
# baseline (speedup 1.0000x reference)
"""Trainium2 Bass kernel for PraxisAttention (differential attention variant).

Reference computation (per batch b, head h):
  q = x @ Wq.T   -> [S, H, 2*Dh]  (Q1 = first Dh, Q2 = second Dh)
  k = x @ Wk.T   -> [S, H, 2*Dh]
  v = x @ Wv.T   -> [S, H, Dh]
  s_i = (Q_i @ K_i^T) / sqrt(Dh) + slope_h*(pos_k - pos_q) + causal + amask
  w_i = softmax(s_i)
  w = w1 - lam * w2 ;  out_h = w @ V_h
  GroupNorm per (b,h) over (S,Dh), * gamma + beta, * (1-0.8)
  out = concat_h @ Wo.T

Sharding: 8 cores = 2 batches x 4 head-groups (4 heads each).  Each core
computes a partial [S, D] output (its heads' contribution to the final
projection); host sums the 4 partials per batch.

Device-side algorithm per core (all matmuls fp32r):
  - x^T loaded [D, S]; Q^T/K^T computed per head as [2Dh, S]; scores are
    computed transposed: s^T[k, q] = K_aug^T.T @ Q_aug^T with the contraction
    dim augmented by 2 rows folding the ALiBi bias in (pos_k*slope - slope*pos_q).
  - padding mask folded into the exp() activation bias (per-partition = per-k).
  - causal mask: upper tiles skipped entirely; diagonal tiles get a
    host-precomputed triangular -1e9 pattern added.
  - P^T = exp(s^T); O^T[dv,q] = sum_k V'[k,dv] P^T[k,q] with V' carrying an
    extra ones-column so row 64 of O^T is the softmax denominator.
  - combine: normed^T = O1^T/d1 - lam*O2^T/d2  (denominator rows broadcast
    across partitions via gpsimd.partition_broadcast).
  - GroupNorm stats via bn_stats/bn_aggr + partition_all_reduce; applied as a
    per-partition affine (gamma/beta/(1-0.8) folded in on host).
  - projection: out[q,:] += normed_pair^T.T @ Wo_pair^T accumulated over the
    two head-pairs in PSUM, DMA'd straight to DRAM.
"""

import numpy as np

import concourse.bass as bass
import concourse.bacc as bacc
import concourse.mybir as mybir
import concourse.tile as tile
from concourse import bass_isa
from concourse.bass_utils import run_bass_kernel_spmd

# Problem constants (hardcoded per contract).
B, S, D = 2, 2048, 1024
H, Dh = 16, 64
LAMBDA_INIT = 0.8
NEG = -1e9
EPS = 1e-5

P = 128            # SBUF partitions
NH = 4             # heads per core
TDH = 2 * Dh       # 128
DC = D // P        # 8 contraction chunks for the projections
SC = S // P        # 16 key chunks of 128
QW = 512           # query tile width
QC = S // QW       # 4 query chunks
KAUG = Dh + 2      # augmented contraction dim for score matmuls
TRIW = 896         # causal triangle tile width (512 + 3*128)

F32 = mybir.dt.float32
F32R = mybir.dt.float32r
AF = mybir.ActivationFunctionType
ALU = mybir.AluOpType


def _emit(nc: bass.Bass, tc, ctx, lam: float):
    xT = nc.dram_tensor("xT", [D, S], F32R, kind="ExternalInput")
    wqT = nc.dram_tensor("wqT", [D, NH * TDH], F32R, kind="ExternalInput")
    wkT = nc.dram_tensor("wkT", [D, NH * TDH], F32R, kind="ExternalInput")
    wvT = nc.dram_tensor("wvT", [D, NH * Dh], F32R, kind="ExternalInput")
    woT = nc.dram_tensor("woT", [NH * Dh, D], F32R, kind="ExternalInput")
    amask = nc.dram_tensor("amask", [S], F32, kind="ExternalInput")
    tri = nc.dram_tensor("tri", [P, TRIW], F32, kind="ExternalInput")
    augk = nc.dram_tensor("augk", [2, S], F32R, kind="ExternalInput")
    augq = nc.dram_tensor("augq", [NH, 2, S], F32R, kind="ExternalInput")
    gnsc = nc.dram_tensor("gnsc", [Dh, NH], F32, kind="ExternalInput")
    gnsh = nc.dram_tensor("gnsh", [Dh, NH], F32, kind="ExternalInput")
    out = nc.dram_tensor("out", [S, D], F32, kind="ExternalOutput")

    const = ctx.enter_context(tc.tile_pool(name="const", bufs=1))
    wpool = ctx.enter_context(tc.tile_pool(name="wpool", bufs=2))
    augp = ctx.enter_context(tc.tile_pool(name="augp", bufs=1))
    vpool = ctx.enter_context(tc.tile_pool(name="vpool", bufs=1))
    pairp = ctx.enter_context(tc.tile_pool(name="pairp", bufs=1))
    ppool = ctx.enter_context(tc.tile_pool(name="ppool", bufs=2))
    rbpool = ctx.enter_context(tc.tile_pool(name="rbpool", bufs=5))
    smallp = ctx.enter_context(tc.tile_pool(name="smallp", bufs=2))
    outp = ctx.enter_context(tc.tile_pool(name="outp", bufs=2))
    psq = ctx.enter_context(tc.tile_pool(name="psq", bufs=2, space="PSUM"))
    pss = ctx.enter_context(tc.tile_pool(name="pss", bufs=2, space="PSUM"))
    pso = ctx.enter_context(tc.tile_pool(name="pso", bufs=2, space="PSUM"))

    # ---- constant loads -------------------------------------------------
    xs = const.tile([P, DC, S], F32R)
    xTr = xT.rearrange("(c p) s -> p c s", p=P)
    for dc in range(DC):
        nc.sync.dma_start(out=xs[:, dc, :], in_=xTr[:, dc, :])

    wv_sb = const.tile([P, DC, NH * Dh], F32R)
    nc.sync.dma_start(out=wv_sb, in_=wvT.rearrange("(c p) n -> p c n", p=P))
    wo_sb = const.tile([P, 2, D], F32R)
    nc.sync.dma_start(out=wo_sb, in_=woT.rearrange("(t p) n -> p t n", p=P))
    tri_sb = const.tile([P, TRIW], F32)
    nc.sync.dma_start(out=tri_sb, in_=tri[:, :])
    amask_sb = const.tile([P, SC], F32)
    nc.sync.dma_start(out=amask_sb, in_=amask.rearrange("(c p) -> p c", p=P))
    gnsc_sb = const.tile([Dh, NH], F32)
    nc.sync.dma_start(out=gnsc_sb, in_=gnsc[:, :])
    gnsh_sb = const.tile([Dh, NH], F32)
    nc.sync.dma_start(out=gnsh_sb, in_=gnsh[:, :])
    eps_sb = const.tile([Dh, 1], F32)
    nc.vector.memset(eps_sb, EPS)
    ones64 = const.tile([P, SC * NH], F32)
    nc.vector.memset(ones64, 1.0)

    # ---- phase B: V' for all heads -------------------------------------
    vp_all = vpool.tile([P, SC, NH, Dh + 1], F32R, name="vp_all", tag="vp_all")
    nc.vector.tensor_copy(
        out=vp_all[:, :, :, Dh],
        in_=ones64.rearrange("p (a b) -> p a b", a=SC),
    )
    for sc in range(SC):
        vpsum = psq.tile([P, NH * Dh], F32, tag="psq", name=f"vpsum{sc}")
        for dc in range(DC):
            nc.tensor.matmul(
                vpsum,
                lhsT=xs[:, dc, sc * P:(sc + 1) * P],
                rhs=wv_sb[:, dc, :],
                start=(dc == 0),
                stop=(dc == DC - 1),
            )
        nc.vector.tensor_copy(
            out=vp_all[:, sc, :, 0:Dh],
            in_=vpsum.rearrange("p (g d) -> p g d", g=NH),
        )

    pairs = []
    for i in range(2):
        pr_t = pairp.tile([P, S], F32R, name=f"pair{i}", tag=f"pair{i}")
        pairs.append(pr_t)

    # ---- phase C: per-head attention -----------------------------------
    for g in range(NH):
        wq_sb = wpool.tile([P, DC, TDH], F32R, tag="wq", name=f"wq{g}")
        nc.sync.dma_start(
            out=wq_sb,
            in_=wqT.rearrange("(c p) (g m) -> p c g m", p=P, m=TDH)[:, :, g, :],
        )
        wk_sb = wpool.tile([P, DC, TDH], F32R, tag="wk", name=f"wk{g}")
        nc.sync.dma_start(
            out=wk_sb,
            in_=wkT.rearrange("(c p) (g m) -> p c g m", p=P, m=TDH)[:, :, g, :],
        )

        q1 = augp.tile([KAUG, S], F32R, tag="q1", name=f"q1_{g}")
        q2 = augp.tile([KAUG, S], F32R, tag="q2", name=f"q2_{g}")
        k1 = augp.tile([KAUG, S], F32R, tag="k1", name=f"k1_{g}")
        k2 = augp.tile([KAUG, S], F32R, tag="k2", name=f"k2_{g}")

        for scq in range(QC):
            sl = slice(scq * QW, (scq + 1) * QW)
            qt = psq.tile([P, QW], F32, tag="psq", name=f"qt{g}_{scq}")
            for dc in range(DC):
                nc.tensor.matmul(
                    qt,
                    lhsT=wq_sb[:, dc, :],
                    rhs=xs[:, dc, sl],
                    start=(dc == 0),
                    stop=(dc == DC - 1),
                )
            nc.vector.tensor_copy(out=q1[0:Dh, sl], in_=qt[0:Dh, :])
            nc.vector.tensor_copy(out=q2[0:Dh, sl], in_=qt[Dh:TDH, :])
            kt = psq.tile([P, QW], F32, tag="psq", name=f"kt{g}_{scq}")
            for dc in range(DC):
                nc.tensor.matmul(
                    kt,
                    lhsT=wk_sb[:, dc, :],
                    rhs=xs[:, dc, sl],
                    start=(dc == 0),
                    stop=(dc == DC - 1),
                )
            nc.vector.tensor_copy(out=k1[0:Dh, sl], in_=kt[0:Dh, :])
            nc.vector.tensor_copy(out=k2[0:Dh, sl], in_=kt[Dh:TDH, :])

        # augmentation rows: Q side carries (slope, -slope*pos_q); K side
        # carries (pos_k, 1).  Written via DMA (compute engines can only
        # address 32-aligned partition starts).
        for qa in (q1, q2):
            nc.sync.dma_start(out=qa[Dh:Dh + 2, :], in_=augq[g, :, :])
        for ka in (k1, k2):
            nc.sync.dma_start(out=ka[Dh:Dh + 2, :], in_=augk[:, :])

        stats = smallp.tile([Dh, QC, 6], F32, tag="stats", name=f"stats{g}")

        for qc in range(QC):
            qsl = slice(qc * QW, (qc + 1) * QW)
            o1 = pso.tile([Dh + 1, QW], F32, tag="pso", name=f"o1_{g}_{qc}")
            o2 = pso.tile([Dh + 1, QW], F32, tag="pso", name=f"o2_{g}_{qc}")
            kmax = 4 * qc + 4
            for kc in range(kmax):
                ksl = slice(kc * P, (kc + 1) * P)
                dlt = kc - 4 * qc
                s12 = pss.tile([P, 2, QW], F32, tag="pss",
                               name=f"s12_{g}_{qc}_{kc}")
                for i, (qa, ka) in enumerate(((q1, k1), (q2, k2))):
                    nc.tensor.matmul(
                        s12[:, i, :],
                        lhsT=ka[:, ksl],
                        rhs=qa[:, qsl],
                        start=True,
                        stop=True,
                    )
                if dlt >= 0:
                    # causal mask: only the first 128*(dlt+1) cols have any
                    # masked rows
                    w = 128 * (dlt + 1)
                    c0 = 384 - 128 * dlt
                    for i in range(2):
                        nc.vector.tensor_add(
                            out=s12[:, i, 0:w], in0=s12[:, i, 0:w],
                            in1=tri_sb[:, c0:c0 + w],
                        )
                p12 = ppool.tile([P, 2, QW], F32R, tag="p",
                                 name=f"p12_{g}_{qc}_{kc}")
                nc.scalar.activation(
                    out=p12, in_=s12, func=AF.Exp,
                    bias=amask_sb[:, kc:kc + 1], scale=1.0,
                )
                for i, oacc in enumerate((o1, o2)):
                    nc.tensor.matmul(
                        oacc,
                        lhsT=vp_all[:, kc, g, :],
                        rhs=p12[:, i, :],
                        start=(kc == 0),
                        stop=(kc == kmax - 1),
                    )

            # drain O psum to sbuf immediately (frees the accumulators for
            # the next q-chunk), then normalize + differential combine
            o1s = rbpool.tile([Dh + 1, QW], F32, tag="os", bufs=4, name=f"o1s_{g}_{qc}")
            nc.vector.tensor_copy(out=o1s, in_=o1)
            o2s = rbpool.tile([Dh + 1, QW], F32, tag="os", bufs=4, name=f"o2s_{g}_{qc}")
            nc.vector.tensor_copy(out=o2s, in_=o2)
            den1 = rbpool.tile([1, QW], F32, tag="den", bufs=4,
                               name=f"den1_{g}_{qc}")
            nc.vector.tensor_copy(out=den1, in_=o1s[Dh:Dh + 1, :])
            den2 = rbpool.tile([1, QW], F32, tag="den", bufs=4,
                               name=f"den2_{g}_{qc}")
            nc.vector.tensor_copy(out=den2, in_=o2s[Dh:Dh + 1, :])
            rb1 = rbpool.tile([Dh, QW], F32, tag="rb", name=f"rb1_{g}_{qc}")
            nc.gpsimd.partition_broadcast(out_ap=rb1, in_ap=den1, channels=Dh)
            nc.vector.reciprocal(out=rb1, in_=rb1)
            rb2 = rbpool.tile([Dh, QW], F32, tag="rb", name=f"rb2_{g}_{qc}")
            nc.gpsimd.partition_broadcast(out_ap=rb2, in_ap=den2, channels=Dh)
            nc.vector.reciprocal(out=rb2, in_=rb2)
            t1 = rbpool.tile([Dh, QW], F32, tag="rb", name=f"t1_{g}_{qc}")
            nc.vector.tensor_mul(out=t1, in0=o1s[0:Dh, :], in1=rb1)
            t2 = rbpool.tile([Dh, QW], F32, tag="rb", name=f"t2_{g}_{qc}")
            nc.vector.tensor_mul(out=t2, in0=o2s[0:Dh, :], in1=rb2)
            prow = pairs[g // 2][64 * (g % 2):64 * (g % 2) + Dh, qsl]
            nc.vector.scalar_tensor_tensor(
                out=prow, in0=t2, scalar=-lam, in1=t1,
                op0=ALU.mult, op1=ALU.add,
            )
            nc.vector.bn_stats(out=stats[:, qc, :], in_=prow)

        # ---- GroupNorm finalize ----
        mv = smallp.tile([Dh, 2], F32, tag="mv", name=f"mv{g}")
        nc.vector.bn_aggr(out=mv, in_=stats)
        red = smallp.tile([Dh, 2], F32, tag="red", name=f"red{g}")
        nc.vector.tensor_mul(out=red[:, 1:2], in0=mv[:, 0:1], in1=mv[:, 0:1])
        nc.vector.tensor_add(out=red[:, 1:2], in0=red[:, 1:2], in1=mv[:, 1:2])
        nc.vector.tensor_copy(out=red[:, 0:1], in_=mv[:, 0:1])
        redo = smallp.tile([Dh, 2], F32, tag="redo", name=f"redo{g}")
        nc.gpsimd.partition_all_reduce(
            out_ap=redo, in_ap=red, channels=Dh, reduce_op=bass_isa.ReduceOp.add
        )
        mstat = smallp.tile([Dh, 4], F32, tag="mstat", name=f"mstat{g}")
        nc.scalar.mul(out=mstat[:, 0:1], in_=redo[:, 0:1], mul=1.0 / Dh)
        nc.scalar.mul(out=mstat[:, 1:2], in_=redo[:, 1:2], mul=1.0 / Dh)
        nc.vector.tensor_mul(out=mstat[:, 3:4], in0=mstat[:, 0:1], in1=mstat[:, 0:1])
        nc.vector.tensor_sub(out=mstat[:, 1:2], in0=mstat[:, 1:2], in1=mstat[:, 3:4])
        nc.scalar.activation(
            out=mstat[:, 2:3], in_=mstat[:, 1:2], func=AF.Sqrt,
            bias=eps_sb, scale=1.0,
        )
        nc.vector.reciprocal(out=mstat[:, 2:3], in_=mstat[:, 2:3])
        sv = smallp.tile([Dh, 2], F32, tag="sv", name=f"sv{g}")
        nc.vector.tensor_mul(out=sv[:, 0:1], in0=mstat[:, 2:3], in1=gnsc_sb[:, g:g + 1])
        nc.vector.tensor_mul(out=sv[:, 1:2], in0=mstat[:, 0:1], in1=sv[:, 0:1])
        nc.vector.tensor_tensor(
            out=sv[:, 1:2], in0=gnsh_sb[:, g:g + 1], in1=sv[:, 1:2],
            op=ALU.subtract,
        )
        rows = pairs[g // 2][64 * (g % 2):64 * (g % 2) + Dh, :]
        nc.vector.tensor_scalar(
            out=rows, in0=rows, scalar1=sv[:, 0:1], scalar2=sv[:, 1:2],
            op0=ALU.mult, op1=ALU.add,
        )

    # ---- phase D: output projection ------------------------------------
    for qs in range(SC):
        psl = slice(qs * P, (qs + 1) * P)
        pab = pss.tile([P, 2, QW], F32, tag="pss", name=f"pab{qs}")
        for half in range(2):
            for pr in range(2):
                nc.tensor.matmul(
                    pab[:, half, :],
                    lhsT=pairs[pr][:, psl],
                    rhs=wo_sb[:, pr, half * QW:(half + 1) * QW],
                    start=(pr == 0),
                    stop=(pr == 1),
                )
        oa = outp.tile([P, D], F32, tag="ob", name=f"oa{qs}")
        nc.vector.tensor_copy(out=oa, in_=pab.rearrange("p a b -> p (a b)"))
        nc.sync.dma_start(out=out[psl, :], in_=oa)


def build_program(lam: float) -> bass.Bass:
    from contextlib import ExitStack

    nc = bacc.Bacc(None, target_bir_lowering=False)
    with tile.TileContext(nc) as tc:
        with ExitStack() as ctx:
            _emit(nc, tc, ctx, lam)
    nc.finalize()
    return nc


def make_in_maps(inputs: dict) -> list[dict]:
    x = np.asarray(inputs["inputs"], dtype=np.float32)
    attention_mask = np.asarray(inputs["attention_mask"], dtype=np.float32)
    token_indices = np.asarray(inputs["token_indices"])
    Wq = np.asarray(inputs["Wq"], dtype=np.float32)
    Wk = np.asarray(inputs["Wk"], dtype=np.float32)
    Wv = np.asarray(inputs["Wv"], dtype=np.float32)
    Wo = np.asarray(inputs["Wo"], dtype=np.float32)
    gn_gamma = np.asarray(inputs["gn_gamma"], dtype=np.float32)
    gn_beta = np.asarray(inputs["gn_beta"], dtype=np.float32)

    sc = 1.0 / np.sqrt(np.float32(Dh))
    WqH = (Wq.reshape(H, TDH, D) * sc).astype(np.float32)   # scale folded into Q
    WkH = Wk.reshape(H, TDH, D)
    WvH = Wv.reshape(H, Dh, D)
    slopes_all = (2.0 ** (-8.0 * np.arange(1, H + 1, dtype=np.float32) / H)).astype(
        np.float32
    )
    gnsc_all = ((1.0 - LAMBDA_INIT) * gn_gamma).reshape(H, Dh).astype(np.float32)
    gnsh_all = ((1.0 - LAMBDA_INIT) * gn_beta).reshape(H, Dh).astype(np.float32)

    kk = np.arange(P, dtype=np.float32)[:, None]
    jj = np.arange(TRIW, dtype=np.float32)[None, :]
    tri_np = np.where(kk > (jj - 384.0), np.float32(NEG), np.float32(0.0)).astype(
        np.float32
    )

    in_maps = []
    for c in range(8):
        b = c // 4
        h0 = NH * (c % 4)
        hs = slice(h0, h0 + NH)
        xT = np.ascontiguousarray(x[b].T)                                # [D, S]
        wqT = np.ascontiguousarray(
            WqH[hs].transpose(2, 0, 1).reshape(D, NH * TDH))
        wkT = np.ascontiguousarray(
            WkH[hs].transpose(2, 0, 1).reshape(D, NH * TDH))
        wvT = np.ascontiguousarray(
            WvH[hs].transpose(2, 0, 1).reshape(D, NH * Dh))
        woT = np.ascontiguousarray(Wo[:, Dh * h0:Dh * (h0 + NH)].T)      # [256, D]
        posb = token_indices[b].astype(np.float32)
        amaskb = ((1.0 - attention_mask[b]) * NEG).astype(np.float32)
        augk_np = np.stack([posb, np.ones(S, np.float32)]).astype(np.float32)
        augq_np = np.empty((NH, 2, S), np.float32)
        for g in range(NH):
            sl_h = slopes_all[h0 + g]
            augq_np[g, 0, :] = sl_h
            augq_np[g, 1, :] = -sl_h * posb
        in_maps.append({
            "xT": xT,
            "wqT": wqT,
            "wkT": wkT,
            "wvT": wvT,
            "woT": woT,
            "amask": amaskb,
            "tri": tri_np,
            "augk": augk_np,
            "augq": augq_np,
            "gnsc": np.ascontiguousarray(gnsc_all[hs].T),                # [Dh, NH]
            "gnsh": np.ascontiguousarray(gnsh_all[hs].T),
        })
    return in_maps


def compute_lam(inputs: dict) -> float:
    lam_q1 = np.asarray(inputs["lam_q1"], dtype=np.float32)
    lam_q2 = np.asarray(inputs["lam_q2"], dtype=np.float32)
    lam_k1 = np.asarray(inputs["lam_k1"], dtype=np.float32)
    lam_k2 = np.asarray(inputs["lam_k2"], dtype=np.float32)
    return float(
        np.exp(np.dot(lam_q1, lam_k1)) - np.exp(np.dot(lam_q2, lam_k2)) + LAMBDA_INIT
    )


def _run(inputs: dict, trace: bool = False, **spmd_kwargs):
    lam = compute_lam(inputs)
    nc = build_program(lam)
    in_maps = make_in_maps(inputs)
    res = run_bass_kernel_spmd(
        nc, in_maps, core_ids=list(range(8)), trace=trace, **spmd_kwargs
    )
    parts = [r["out"] for r in res.results]
    out = np.empty((B, S, D), dtype=np.float32)
    for b in range(B):
        out[b] = parts[4 * b] + parts[4 * b + 1] + parts[4 * b + 2] + parts[4 * b + 3]
    return out, res


def kernel(**inputs) -> np.ndarray:
    out, _ = _run(inputs)
    return out


# revision 33
# speedup vs baseline: 278.6129x; 278.6129x over previous
"""Trainium2 Bass kernel for PraxisAttention (differential attention variant).

Reference computation (per batch b, head h):
  q = x @ Wq.T   -> [S, H, 2*Dh]  (Q1 = first Dh, Q2 = second Dh)
  k = x @ Wk.T   -> [S, H, 2*Dh]
  v = x @ Wv.T   -> [S, H, Dh]
  s_i = (Q_i @ K_i^T) / sqrt(Dh) + slope_h*(pos_k - pos_q) + causal + amask
  w_i = softmax(s_i)
  w = w1 - lam * w2 ;  out_h = w @ V_h
  GroupNorm per (b,h) over (S,Dh), * gamma + beta, * (1-0.8)
  out = concat_h @ Wo.T

Sharding: 8 cores = 2 batches x 4 head-groups (4 heads each).  Each core
computes a partial [S, D] output (its heads' contribution to the final
projection); host sums the 4 partials per batch.

Device-side algorithm per core (all matmuls fp32r):
  - x^T loaded [D, S]; Q^T/K^T computed per head as [2Dh, S]; scores are
    computed transposed: s^T[k, q] = K_aug^T.T @ Q_aug^T with the contraction
    dim augmented by 2 rows folding the ALiBi bias in (pos_k*slope - slope*pos_q).
  - padding mask folded into the exp() activation bias (per-partition = per-k).
  - causal mask: upper tiles skipped entirely; diagonal tiles get a
    host-precomputed triangular -1e9 pattern added.
  - P^T = exp(s^T); O^T[dv,q] = sum_k V'[k,dv] P^T[k,q] with V' carrying an
    extra ones-column so row 64 of O^T is the softmax denominator.
  - combine: normed^T = O1^T/d1 - lam*O2^T/d2  (denominator rows broadcast
    across partitions via gpsimd.partition_broadcast).
  - GroupNorm stats via bn_stats/bn_aggr + partition_all_reduce; applied as a
    per-partition affine (gamma/beta/(1-0.8) folded in on host).
  - projection: out[q,:] += normed_pair^T.T @ Wo_pair^T accumulated over the
    two head-pairs in PSUM, DMA'd straight to DRAM.
"""

import numpy as np

import concourse.bass as bass
import concourse.bacc as bacc
import concourse.mybir as mybir
import concourse.tile as tile
from concourse import bass_isa
from concourse.bass_utils import run_bass_kernel_spmd

# Problem constants (hardcoded per contract).
B, S, D = 2, 2048, 1024
H, Dh = 16, 64
LAMBDA_INIT = 0.8
NEG = -1e9
EPS = 1e-5

P = 128            # SBUF partitions
NH = 4             # heads per core
TDH = 2 * Dh       # 128
DC = D // P        # 8 contraction chunks for the projections
SC = S // P        # 16 key chunks of 128
QW = 512           # query tile width
QC = S // QW       # 4 query chunks
KAUG = Dh + 2      # augmented contraction dim for score matmuls
TRIW = 512         # causal triangle tile width

F32 = mybir.dt.float32
F32R = mybir.dt.float32r
AF = mybir.ActivationFunctionType
ALU = mybir.AluOpType


def _declare_io(nc: bass.Bass):
    io = {}
    io["xT"] = nc.dram_tensor("xT", [D, S], F32R, kind="ExternalInput")
    io["wqT"] = nc.dram_tensor("wqT", [D, NH * TDH], F32R, kind="ExternalInput")
    io["wkT"] = nc.dram_tensor("wkT", [D, NH * TDH], F32R, kind="ExternalInput")
    io["wvT"] = nc.dram_tensor("wvT", [D, NH * Dh], F32R, kind="ExternalInput")
    io["woT"] = nc.dram_tensor("woT", [NH * Dh, D], F32R, kind="ExternalInput")
    io["amask"] = nc.dram_tensor("amask", [S], F32, kind="ExternalInput")
    io["tri"] = nc.dram_tensor("tri", [P, TRIW], F32, kind="ExternalInput")
    io["augk"] = nc.dram_tensor("augk", [2, S], F32R, kind="ExternalInput")
    io["augq"] = nc.dram_tensor("augq", [NH, 2, S], F32R, kind="ExternalInput")
    io["gnsc"] = nc.dram_tensor("gnsc", [Dh, NH], F32, kind="ExternalInput")
    io["gnsh"] = nc.dram_tensor("gnsh", [Dh, NH], F32, kind="ExternalInput")
    io["out"] = nc.dram_tensor("out", [S, D], F32, kind="ExternalOutput")
    return io


def _emit(nc: bass.Bass, tc, ctx, lam: float, io: dict):
    xT = io["xT"]; wqT = io["wqT"]; wkT = io["wkT"]; wvT = io["wvT"]
    woT = io["woT"]; amask = io["amask"]; tri = io["tri"]
    augk = io["augk"]; augq = io["augq"]; gnsc = io["gnsc"]; gnsh = io["gnsh"]
    out = io["out"]

    const = ctx.enter_context(tc.tile_pool(name="const", bufs=1))
    wpool = ctx.enter_context(tc.tile_pool(name="wpool", bufs=2))
    augp = ctx.enter_context(tc.tile_pool(name="augp", bufs=1))
    vpool = ctx.enter_context(tc.tile_pool(name="vpool", bufs=1))
    pairp = ctx.enter_context(tc.tile_pool(name="pairp", bufs=1))
    ppool = ctx.enter_context(tc.tile_pool(name="ppool", bufs=2))
    rbpool = ctx.enter_context(tc.tile_pool(name="rbpool", bufs=4))
    smallp = ctx.enter_context(tc.tile_pool(name="smallp", bufs=2))
    outp = ctx.enter_context(tc.tile_pool(name="outp", bufs=2))
    pss = ctx.enter_context(tc.tile_pool(name="pss", bufs=3, space="PSUM"))
    pso = ctx.enter_context(tc.tile_pool(name="pso", bufs=2, space="PSUM"))

    # ---- constant loads -------------------------------------------------
    xs = const.tile([P, DC, S], F32R)
    xTr = xT.rearrange("(c p) s -> p c s", p=P)
    dma_engines = [nc.sync, nc.gpsimd, nc.scalar]
    for dc in range(DC):
        dma_engines[dc % 3].dma_start(out=xs[:, dc, :], in_=xTr[:, dc, :])

    wv_sb = const.tile([P, DC, NH * Dh], F32R, tag="wvo", padded_shape=[P, DC, NH * Dh])
    nc.sync.dma_start(out=wv_sb, in_=wvT.rearrange("(c p) n -> p c n", p=P))
    tri_sb = const.tile([P, TRIW], F32)
    nc.gpsimd.dma_start(out=tri_sb, in_=tri[:, :])
    amask_sb = const.tile([P, SC], F32)
    nc.scalar.dma_start(out=amask_sb, in_=amask.rearrange("(c p) -> p c", p=P))
    gnsc_sb = const.tile([Dh, NH], F32)
    nc.scalar.dma_start(out=gnsc_sb, in_=gnsc[:, :])
    gnsh_sb = const.tile([Dh, NH], F32)
    nc.scalar.dma_start(out=gnsh_sb, in_=gnsh[:, :])
    eps_sb = const.tile([Dh, 1], F32)
    nc.vector.memset(eps_sb, EPS)
    ones64 = const.tile([P, SC * NH], F32)
    nc.vector.memset(ones64, 1.0)
    ones_r = const.tile([Dh, Dh], F32R)
    nc.vector.tensor_copy(out=ones_r, in_=ones64[0:Dh, 0:Dh])

    # ---- phase B: V' for all heads -------------------------------------
    vp_all = vpool.tile([P, SC, NH, Dh + 1], F32R, name="vp_all", tag="vp_all")
    nc.vector.tensor_copy(
        out=vp_all[:, :, :, Dh],
        in_=ones64.rearrange("p (a b) -> p a b", a=SC),
    )
    for sc in range(SC):
        vpsum = pss.tile([P, NH * Dh], F32, tag="pss", name=f"vpsum{sc}")
        for dc in range(DC):
            nc.tensor.matmul(
                vpsum,
                lhsT=xs[:, dc, sc * P:(sc + 1) * P],
                rhs=wv_sb[:, dc, :],
                start=(dc == 0),
                stop=(dc == DC - 1),
            )
        nc.vector.tensor_copy(
            out=vp_all[:, sc, :, 0:Dh],
            in_=vpsum.rearrange("p (g d) -> p g d", g=NH),
        )

    wo_sb = const.tile([P, 2, D], F32R, tag="wvo", padded_shape=[P, 2, D])
    nc.sync.dma_start(out=wo_sb, in_=woT.rearrange("(t p) n -> p t n", p=P))

    pairs = []
    for i in range(2):
        pr_t = pairp.tile([P, S], F32R, name=f"pair{i}", tag=f"pair{i}")
        pairs.append(pr_t)

    # K-side aug rows (pos_k, 1) are head-independent: two sets (even/odd
    # heads) so head g+1's K-projection copies don't wait on head g's last
    # score matmuls.
    k_sets = []
    for par in range(2):
        ka_ = augp.tile([KAUG, S], F32R, tag=f"k1_{par}", name=f"k1_{par}")
        kb_ = augp.tile([KAUG, S], F32R, tag=f"k2_{par}", name=f"k2_{par}")
        nc.sync.dma_start(out=ka_[Dh:Dh + 2, :], in_=augk[:, :])
        nc.gpsimd.dma_start(out=kb_[Dh:Dh + 2, :], in_=augk[:, :])
        k_sets.append((ka_, kb_))

    # ---- phase C: per-head attention -----------------------------------
    redosbs = []
    for g in range(NH):
        wq_sb = wpool.tile([P, DC, TDH], F32R, tag="wq", name=f"wq{g}")
        nc.sync.dma_start(
            out=wq_sb,
            in_=wqT.rearrange("(c p) (g m) -> p c g m", p=P, m=TDH)[:, :, g, :],
        )
        wk_sb = wpool.tile([P, DC, TDH], F32R, tag="wk", name=f"wk{g}")
        nc.sync.dma_start(
            out=wk_sb,
            in_=wkT.rearrange("(c p) (g m) -> p c g m", p=P, m=TDH)[:, :, g, :],
        )

        q1 = augp.tile([KAUG, S], F32R, tag="q1", name=f"q1_{g}")
        q2 = augp.tile([KAUG, S], F32R, tag="q2", name=f"q2_{g}")
        k1, k2 = k_sets[g % 2]

        for scq in range(QC):
            sl = slice(scq * QW, (scq + 1) * QW)
            qt = pss.tile([P, QW], F32, tag="pss", name=f"qt{g}_{scq}")
            for dc in range(DC):
                nc.tensor.matmul(
                    qt,
                    lhsT=wq_sb[:, dc, :],
                    rhs=xs[:, dc, sl],
                    start=(dc == 0),
                    stop=(dc == DC - 1),
                )
            nc.vector.tensor_copy(out=q1[0:Dh, sl], in_=qt[0:Dh, :])
            nc.vector.tensor_copy(out=q2[0:Dh, sl], in_=qt[Dh:TDH, :])
            kt = pss.tile([P, QW], F32, tag="pss", name=f"kt{g}_{scq}")
            for dc in range(DC):
                nc.tensor.matmul(
                    kt,
                    lhsT=wk_sb[:, dc, :],
                    rhs=xs[:, dc, sl],
                    start=(dc == 0),
                    stop=(dc == DC - 1),
                )
            nc.scalar.copy(out=k1[0:Dh, sl], in_=kt[0:Dh, :])
            nc.scalar.copy(out=k2[0:Dh, sl], in_=kt[Dh:TDH, :])

        # augmentation rows: Q side carries (slope, -slope*pos_q); K side
        # carries (pos_k, 1).  Written via DMA (compute engines can only
        # address 32-aligned partition starts).
        for qa in (q1, q2):
            nc.sync.dma_start(out=qa[Dh:Dh + 2, :], in_=augq[g, :, :])

        stats = smallp.tile([Dh, QC, 6], F32, tag="stats", name=f"stats{g}")

        for qc in range(QC):
            qsl = slice(qc * QW, (qc + 1) * QW)
            o1 = pso.tile([Dh + 1, QW], F32, tag="pso", name=f"o1_{g}_{qc}")
            o2 = pso.tile([Dh + 1, QW], F32, tag="pso", name=f"o2_{g}_{qc}")
            kmax = 4 * qc + 4
            for kc in range(kmax):
                ksl = slice(kc * P, (kc + 1) * P)
                dlt = kc - 4 * qc
                s12 = pss.tile([P, 2, QW], F32, tag="pss",
                               name=f"s12_{g}_{qc}_{kc}")
                for i, (qa, ka) in enumerate(((q1, k1), (q2, k2))):
                    nc.tensor.matmul(
                        s12[:, i, :],
                        lhsT=ka[:, ksl],
                        rhs=qa[:, qsl],
                        start=True,
                        stop=True,
                    )
                if dlt >= 0:
                    # causal mask: only the first 128*(dlt+1) cols have any
                    # masked rows
                    w = 128 * (dlt + 1)
                    c0 = 384 - 128 * dlt
                    for i in range(2):
                        nc.vector.tensor_add(
                            out=s12[:, i, 0:w], in0=s12[:, i, 0:w],
                            in1=tri_sb[:, c0:c0 + w],
                        )
                p12 = ppool.tile([P, 2, QW], F32R, tag="p",
                                 name=f"p12_{g}_{qc}_{kc}")
                nc.scalar.activation(
                    out=p12, in_=s12, func=AF.Exp,
                    bias=amask_sb[:, kc:kc + 1], scale=1.0,
                )
                for i, oacc in enumerate((o1, o2)):
                    nc.tensor.matmul(
                        oacc,
                        lhsT=vp_all[:, kc, g, :],
                        rhs=p12[:, i, :],
                        start=(kc == 0),
                        stop=(kc == kmax - 1),
                    )

            # drain O psum to sbuf immediately (frees the accumulators for
            # the next q-chunk), then normalize + differential combine
            o1s = rbpool.tile([Dh + 1, QW], F32, tag="os", bufs=3, name=f"o1s_{g}_{qc}")
            nc.vector.tensor_copy(out=o1s, in_=o1)
            o2s = rbpool.tile([Dh + 1, QW], F32, tag="os", bufs=3, name=f"o2s_{g}_{qc}")
            nc.vector.tensor_copy(out=o2s, in_=o2)
            den1 = rbpool.tile([1, QW], F32, tag="den", bufs=3,
                               name=f"den1_{g}_{qc}")
            nc.vector.tensor_copy(out=den1, in_=o1s[Dh:Dh + 1, :])
            nc.vector.reciprocal(out=den1, in_=den1)
            den2 = rbpool.tile([1, QW], F32, tag="den", bufs=3,
                               name=f"den2_{g}_{qc}")
            # scale by -1/lam so its reciprocal is -lam/d2 (folds the
            # differential combine into the normalizer)
            nc.vector.tensor_scalar_mul(
                out=den2, in0=o2s[Dh:Dh + 1, :], scalar1=-1.0 / lam)
            nc.vector.reciprocal(out=den2, in_=den2)
            rb1 = rbpool.tile([Dh, QW], F32, tag="rb", name=f"rb1_{g}_{qc}")
            nc.gpsimd.partition_broadcast(out_ap=rb1, in_ap=den1, channels=Dh)
            rb2 = rbpool.tile([Dh, QW], F32, tag="rb", name=f"rb2_{g}_{qc}")
            nc.gpsimd.partition_broadcast(out_ap=rb2, in_ap=den2, channels=Dh)
            t1 = rbpool.tile([Dh, QW], F32, tag="rb", name=f"t1_{g}_{qc}")
            nc.gpsimd.tensor_mul(out=t1, in0=o1s[0:Dh, :], in1=rb1)
            t2 = rbpool.tile([Dh, QW], F32, tag="rb", name=f"t2_{g}_{qc}")
            nc.gpsimd.tensor_mul(out=t2, in0=o2s[0:Dh, :], in1=rb2)
            prow = pairs[g // 2][64 * (g % 2):64 * (g % 2) + Dh, qsl]
            nc.gpsimd.tensor_add(out=prow, in0=t1, in1=t2)
            nc.vector.bn_stats(out=stats[:, qc, :], in_=prow)

        # ---- GroupNorm stats reduce (finalize deferred past the loop so
        # the Sqrt doesn't force Exp-table reloads on ACT mid-stream) ----
        mv = smallp.tile([Dh, 2], F32, tag="mv", name=f"mv{g}")
        nc.vector.bn_aggr(out=mv, in_=stats)
        red = smallp.tile([Dh, 2], F32R, tag="red", name=f"red{g}")
        nc.vector.tensor_mul(out=red[:, 1:2], in0=mv[:, 0:1], in1=mv[:, 0:1])
        nc.vector.tensor_add(out=red[:, 1:2], in0=red[:, 1:2], in1=mv[:, 1:2])
        nc.vector.tensor_copy(out=red[:, 0:1], in_=mv[:, 0:1])
        redo = pso.tile([Dh, 2], F32, tag="pso", name=f"redo{g}")
        nc.tensor.matmul(redo, lhsT=ones_r, rhs=red, start=True, stop=True)
        redosb = smallp.tile([Dh, 2], F32, tag=f"redosb{g}", name=f"redosb{g}")
        nc.vector.tensor_copy(out=redosb, in_=redo)
        redosbs.append(redosb)

    # ---- GroupNorm finalize (all heads) --------------------------------
    for g in range(NH):
        redosb = redosbs[g]
        mstat = smallp.tile([Dh, 4], F32, tag="mstat", name=f"mstat{g}")
        nc.scalar.mul(out=mstat[:, 0:1], in_=redosb[:, 0:1], mul=1.0 / Dh)
        nc.scalar.mul(out=mstat[:, 1:2], in_=redosb[:, 1:2], mul=1.0 / Dh)
        nc.vector.tensor_mul(out=mstat[:, 3:4], in0=mstat[:, 0:1], in1=mstat[:, 0:1])
        nc.vector.tensor_sub(out=mstat[:, 1:2], in0=mstat[:, 1:2], in1=mstat[:, 3:4])
        nc.scalar.activation(
            out=mstat[:, 2:3], in_=mstat[:, 1:2], func=AF.Sqrt,
            bias=eps_sb, scale=1.0,
        )
        nc.vector.reciprocal(out=mstat[:, 2:3], in_=mstat[:, 2:3])
        sv = smallp.tile([Dh, 2], F32, tag="sv", name=f"sv{g}")
        nc.vector.tensor_mul(out=sv[:, 0:1], in0=mstat[:, 2:3], in1=gnsc_sb[:, g:g + 1])
        nc.vector.tensor_mul(out=sv[:, 1:2], in0=mstat[:, 0:1], in1=sv[:, 0:1])
        nc.vector.tensor_tensor(
            out=sv[:, 1:2], in0=gnsh_sb[:, g:g + 1], in1=sv[:, 1:2],
            op=ALU.subtract,
        )
        rows = pairs[g // 2][64 * (g % 2):64 * (g % 2) + Dh, :]
        nc.vector.tensor_scalar(
            out=rows, in0=rows, scalar1=sv[:, 0:1], scalar2=sv[:, 1:2],
            op0=ALU.mult, op1=ALU.add,
        )

    # ---- phase D: output projection ------------------------------------
    for qs in range(SC):
        psl = slice(qs * P, (qs + 1) * P)
        pab = pss.tile([P, 2, QW], F32, tag="pss", name=f"pab{qs}")
        for half in range(2):
            for pr in range(2):
                nc.tensor.matmul(
                    pab[:, half, :],
                    lhsT=pairs[pr][:, psl],
                    rhs=wo_sb[:, pr, half * QW:(half + 1) * QW],
                    start=(pr == 0),
                    stop=(pr == 1),
                )
        oa = outp.tile([P, D], F32, tag="ob", name=f"oa{qs}")
        nc.vector.tensor_copy(out=oa[:, 0:QW], in_=pab[:, 0, :])
        nc.scalar.copy(out=oa[:, QW:D], in_=pab[:, 1, :])
        dma_engines[qs % 3].dma_start(out=out[psl, :], in_=oa)


def build_program(lam: float, reps: int = 1) -> bass.Bass:
    from contextlib import ExitStack

    nc = bacc.Bacc(None, target_bir_lowering=False)
    with tile.TileContext(nc) as tc:
        io = _declare_io(nc)
        for _ in range(reps):
            with ExitStack() as ctx:
                _emit(nc, tc, ctx, lam, io)
    nc.finalize()
    return nc


def make_in_maps(inputs: dict) -> list[dict]:
    x = np.asarray(inputs["inputs"], dtype=np.float32)
    attention_mask = np.asarray(inputs["attention_mask"], dtype=np.float32)
    token_indices = np.asarray(inputs["token_indices"])
    Wq = np.asarray(inputs["Wq"], dtype=np.float32)
    Wk = np.asarray(inputs["Wk"], dtype=np.float32)
    Wv = np.asarray(inputs["Wv"], dtype=np.float32)
    Wo = np.asarray(inputs["Wo"], dtype=np.float32)
    gn_gamma = np.asarray(inputs["gn_gamma"], dtype=np.float32)
    gn_beta = np.asarray(inputs["gn_beta"], dtype=np.float32)

    sc = 1.0 / np.sqrt(np.float32(Dh))
    WqH = (Wq.reshape(H, TDH, D) * sc).astype(np.float32)   # scale folded into Q
    WkH = Wk.reshape(H, TDH, D)
    WvH = Wv.reshape(H, Dh, D)
    slopes_all = (2.0 ** (-8.0 * np.arange(1, H + 1, dtype=np.float32) / H)).astype(
        np.float32
    )
    gnsc_all = ((1.0 - LAMBDA_INIT) * gn_gamma).reshape(H, Dh).astype(np.float32)
    gnsh_all = ((1.0 - LAMBDA_INIT) * gn_beta).reshape(H, Dh).astype(np.float32)

    kk = np.arange(P, dtype=np.float32)[:, None]
    jj = np.arange(TRIW, dtype=np.float32)[None, :]
    tri_np = np.where(kk > (jj - 384.0), np.float32(NEG), np.float32(0.0)).astype(
        np.float32
    )

    in_maps = []
    for c in range(8):
        b = c // 4
        h0 = NH * (c % 4)
        hs = slice(h0, h0 + NH)
        xT = np.ascontiguousarray(x[b].T)                                # [D, S]
        wqT = np.ascontiguousarray(
            WqH[hs].transpose(2, 0, 1).reshape(D, NH * TDH))
        wkT = np.ascontiguousarray(
            WkH[hs].transpose(2, 0, 1).reshape(D, NH * TDH))
        wvT = np.ascontiguousarray(
            WvH[hs].transpose(2, 0, 1).reshape(D, NH * Dh))
        woT = np.ascontiguousarray(Wo[:, Dh * h0:Dh * (h0 + NH)].T)      # [256, D]
        posb = token_indices[b].astype(np.float32)
        amaskb = ((1.0 - attention_mask[b]) * NEG).astype(np.float32)
        augk_np = np.stack([posb, np.ones(S, np.float32)]).astype(np.float32)
        augq_np = np.empty((NH, 2, S), np.float32)
        for g in range(NH):
            sl_h = slopes_all[h0 + g]
            augq_np[g, 0, :] = sl_h
            augq_np[g, 1, :] = -sl_h * posb
        in_maps.append({
            "xT": xT,
            "wqT": wqT,
            "wkT": wkT,
            "wvT": wvT,
            "woT": woT,
            "amask": amaskb,
            "tri": tri_np,
            "augk": augk_np,
            "augq": augq_np,
            "gnsc": np.ascontiguousarray(gnsc_all[hs].T),                # [Dh, NH]
            "gnsh": np.ascontiguousarray(gnsh_all[hs].T),
        })
    return in_maps


def compute_lam(inputs: dict) -> float:
    lam_q1 = np.asarray(inputs["lam_q1"], dtype=np.float32)
    lam_q2 = np.asarray(inputs["lam_q2"], dtype=np.float32)
    lam_k1 = np.asarray(inputs["lam_k1"], dtype=np.float32)
    lam_k2 = np.asarray(inputs["lam_k2"], dtype=np.float32)
    return float(
        np.exp(np.dot(lam_q1, lam_k1)) - np.exp(np.dot(lam_q2, lam_k2)) + LAMBDA_INIT
    )


def _run(inputs: dict, trace: bool = False, **spmd_kwargs):
    lam = compute_lam(inputs)
    nc = build_program(lam)
    in_maps = make_in_maps(inputs)
    res = run_bass_kernel_spmd(
        nc, in_maps, core_ids=list(range(8)), trace=trace, **spmd_kwargs
    )
    parts = [r["out"] for r in res.results]
    out = np.empty((B, S, D), dtype=np.float32)
    for b in range(B):
        out[b] = parts[4 * b] + parts[4 * b + 1] + parts[4 * b + 2] + parts[4 * b + 3]
    return out, res


def kernel(**inputs) -> np.ndarray:
    out, _ = _run(inputs)
    return out
